# revision 1
# baseline (speedup 1.0000x reference)
"""Trainium2 Bass kernel for nn_Attention (Bahdanau-style attention decode step).

Reference computation (per batch b):
    h_proj  = hidden[b] @ W_h                      # [1, H]
    e_proj  = enc[b] @ W_e                         # [S, H]
    energy  = tanh(e_proj + h_proj + bias)         # [S, H]
    attn    = energy @ v                           # [S]
    w       = softmax(attn)                        # [S]
    context = w @ enc[b]                           # [E]

Sharding: data-parallel over batch on 8 cores (4 batches/core), no collectives.

Per-core kernel strategy:
  - enc tiles DMA'd in natural [s=128p, e] layout (contiguous rows) on the
    Pool queue; kept in fp32 for the context matmul's accuracy.
  - DVE converts each enc tile to fp8e4m3 with the (e, e+512) pair packing:
    nat8[s, q, 0] = enc[s, q], nat8[s, q, 1] = enc[s, 512+q], so each uint16
    cell holds one contraction PAIR for the DoubleRow matmul.
  - XBAR DMA transposes (SP queue, 2-byte granules) turn nat8-as-uint16
    [s=128p, q] into encT16 [q=128p, s] — no PE or DVE time spent on the
    transpose at all.
  - Main matmul runs in fp8 DoubleRow mode (0.5 cycles/row, 2x fp32r):
    stationary w_e8[:, t2, :, 64h-block] is [128, 2, 64], moving is the
    byte-strided pair view of encT16, out is a [64, 256] PSUM quadrant.
    The (h_proj + bias) term folds into the tanh via ACT's bias operand.
  - attention = v.T @ energyT on PE (fp32r, full-rate at 512 moving rows).
  - exp on ACT with accum_out accumulating the softmax denominator for free.
    Max-subtraction is skipped: |logits| <= sum|v| ~ 16, safe in fp32.
  - exp rows transposed back to [s=128p] columns on PE (tiny); context
    accumulates on PE against the natural-layout fp32 enc tiles.
  - Pipeline: chunk c+1's load/convert/transpose chain is emitted during
    chunk c's compute; the exp/softmax/context chain for chunk c is
    drained inside chunk c+1's h-loop so PE never head-blocks.
"""

import numpy as np

HIDDEN = 1024
ENC = 1024
BATCH = 32
SEQ = 2048
NCORES = 8
B_LOC = BATCH // NCORES  # 4

S_CHUNK = 512
N_CHUNK = SEQ // S_CHUNK  # 4
SUBS = S_CHUNK // 128  # 4 s-subtiles per chunk
ET = ENC // 128  # 8 e-tiles
ET2 = ENC // 256  # 4 e-pair-tiles (DoubleRow)
HT = HIDDEN // 128  # 8 h-tiles

_CACHED_NC = None


def build_bass(b_loc=B_LOC, seq=SEQ, repeat=1, ablate=(), tune=None, debug=False):
    ablate = set(ablate)
    T = {
        "drain_exp_tp": 4,  # h64-tile index: first exp-transpose drain (4 singles)
        "drain_ctx": 10,    # h64-tile index: drain ctx matmuls (stage B)
        "attn_lag": 2,      # h64-tiles between tanh and its attention matmul
        "lookahead": 2,     # chunks prepared ahead of compute
        "nat_bufs": 4,
        "encT_bufs": 8,
        "energyT_bufs": 6,
        "main_bufs": 2,
        "tp_bufs": 2,
        "attn_bufs": 2,
    }
    T.update(tune or {})
    import concourse.mybir as mybir
    import concourse.tile as tile
    from concourse import bacc
    from concourse.bass import ts
    from concourse.masks import make_identity

    n_chunk = seq // S_CHUNK

    nc = bacc.Bacc()
    R = mybir.dt.float32r
    F = mybir.dt.float32
    F8 = mybir.dt.float8e4
    U16 = mybir.dt.uint16
    AF = mybir.ActivationFunctionType
    DR = mybir.MatmulPerfMode.DoubleRow
    BF = mybir.dt.bfloat16

    hidden = nc.dram_tensor("hidden", [b_loc, HIDDEN], R, kind="ExternalInput")
    enc = nc.dram_tensor("enc", [b_loc, seq, ENC], R, kind="ExternalInput")
    attn_w = nc.dram_tensor("attn_w", [HIDDEN + ENC, HIDDEN], R, kind="ExternalInput")
    attn_b = nc.dram_tensor("attn_b", [HIDDEN], F, kind="ExternalInput")
    v_w = nc.dram_tensor("v_w", [HIDDEN], R, kind="ExternalInput")
    out = nc.dram_tensor("out", [b_loc, ENC], F, kind="ExternalOutput")
    # fp8 staging for the XBAR transposes; slots rotate across chunks
    scratch8 = nc.dram_tensor("scratch8", [4, S_CHUNK, ENC], mybir.dt.float8e4)
    if debug:
        n_chunk_dbg = seq // S_CHUNK
        dbg_et = nc.dram_tensor(
            "dbg_et", [2 * HT, 64, S_CHUNK], F, kind="ExternalOutput"
        )
        dbg_attn = nc.dram_tensor(
            "dbg_attn", [n_chunk_dbg, 2, S_CHUNK], F, kind="ExternalOutput"
        )
        dbg_exp = nc.dram_tensor(
            "dbg_exp", [n_chunk_dbg, 2, S_CHUNK], F, kind="ExternalOutput"
        )
        dbg_encT_early = nc.dram_tensor(
            "dbg_encT_early", [ET2, 128, S_CHUNK], mybir.dt.uint16,
            kind="ExternalOutput",
        )
        dbg_encT_late = nc.dram_tensor(
            "dbg_encT_late", [ET2, 128, S_CHUNK], mybir.dt.uint16,
            kind="ExternalOutput",
        )

    with tile.TileContext(nc) as tc:
        with (
            tc.tile_pool(name="weights", bufs=1) as w_pool,
            tc.tile_pool(name="consts", bufs=1) as const_pool,
            tc.tile_pool(name="nat", bufs=T["nat_bufs"]) as nat_pool,
            tc.tile_pool(name="nat8", bufs=3) as nat8_pool,
            tc.tile_pool(name="encT", bufs=T["encT_bufs"]) as encT_pool,
            tc.tile_pool(name="energyT", bufs=T["energyT_bufs"]) as energyT_pool,
            tc.tile_pool(name="small", bufs=8) as small_pool,
            tc.tile_pool(name="mid", bufs=3) as mid_pool,
            tc.tile_pool(name="ps_tp", bufs=T["tp_bufs"], space="PSUM") as tp_pool,
            tc.tile_pool(name="ps_main", bufs=T["main_bufs"], space="PSUM") as main_pool,
            tc.tile_pool(name="ps_attn", bufs=T["attn_bufs"], space="PSUM") as attn_pool,
            tc.tile_pool(name="ps_ctx", bufs=1, space="PSUM") as ctx_pool,
        ):
            # ---- constants / weights ----
            ident_f = const_pool.tile([128, 128], F, tag="ident_f")
            make_identity(nc, ident_f[:])
            ident = const_pool.tile([128, 128], R)
            nc.vector.tensor_copy(ident[:], ident_f[:])
            ident8 = const_pool.tile([128, 128], F8, tag="ident8")
            nc.vector.tensor_copy(ident8[:], ident_f[:])

            w_h = w_pool.tile([128, ET, HIDDEN], R, tag="w_h")
            nc.sync.dma_start(
                w_h[:], attn_w[0:HIDDEN, :].rearrange("(j p) h -> p j h", p=128)
            )

            # W_e in fp8 with (e, e+512) pairing for DoubleRow:
            #   w_e8[p, t2, i, h] = W_e[i*512 + t2*128 + p, h]
            w_e8 = w_pool.tile([128, ET2, 2, HIDDEN], F8, tag="w_e8")
            we_re = attn_w[HIDDEN : HIDDEN + ENC, :].rearrange(
                "(two t p) h -> p t two h", two=2, t=ET2, p=128
            )
            for t2 in range(ET2):
                wstage = w_pool.tile([128, 2, HIDDEN], R, tag="wstage", name="wstage")
                nc.sync.dma_start(wstage[:], we_re[:, t2, :, :])
                nc.vector.tensor_copy(w_e8[:, t2, :, :], wstage[:])

            # ones row for rank-1 partition-broadcast matmuls
            ones_f = const_pool.tile([1, 128], F, tag="ones_f")
            nc.vector.memset(ones_f[:], 1.0)
            ones_r = const_pool.tile([1, 128], R, tag="ones_r")
            nc.vector.tensor_copy(ones_r[:], ones_f[:])

            # v as [h=64p, i64] columns, duplicated into 2 cols so the
            # attention row lands on 2 PSUM partitions
            vT64 = const_pool.tile([64, 2 * HT, 2], R, tag="vT64")
            for k in range(2):
                nc.gpsimd.dma_start(
                    out=vT64[:, :, k], in_=v_w[:].rearrange("(i p) -> p i", p=64)
                )

            # ---- preamble: hb[b, h] = hidden[b] @ W_h + attn_b, laid out as
            # hbT64 [h=64p, i, b] per-partition bias columns ----
            # NOTE: PE fp32r transposes ahead of DoubleRow matmuls corrupt the
            # DR moving fetch on HW (first 4 of every 16 s-columns), so all
            # preamble transposes are done as (slow, but loop-external)
            # non-contiguous DMA gathers instead.
            hiddenT = const_pool.tile([128, ET, b_loc], R, tag="hiddenT")
            with nc.allow_non_contiguous_dma("tiny preamble gather"):
                for b in range(b_loc):
                    nc.gpsimd.dma_start(
                        out=hiddenT[:, :, b],
                        in_=hidden[b, :].rearrange("(j p) -> p j", p=128),
                    )

            attnb_sb = const_pool.tile([b_loc, HIDDEN], F, tag="attnb")
            nc.gpsimd.dma_start(
                out=attnb_sb[:], in_=attn_b[:].partition_broadcast(b_loc)
            )
            hb_ps = ctx_pool.tile([b_loc, HIDDEN], F, tag="ctx")
            for n in range(2):
                for j in range(ET):
                    nc.tensor.matmul(
                        hb_ps[:, ts(n, 512)],
                        hiddenT[:, j, :],
                        w_h[:, j, ts(n, 512)],
                        start=(j == 0),
                        stop=(j == ET - 1),
                    )
            hb_nat = const_pool.tile([b_loc, HIDDEN], R, tag="hb_nat")
            nc.vector.tensor_add(hb_nat[:], hb_ps[:], attnb_sb[:])

            # per-h64-tile bias columns at partitions 0-63 (DR outputs live
            # there, so the tanh bias operand must too); via DRAM roundtrip +
            # non-contiguous gather, not PE transposes (see note above)
            hb_scratch = nc.dram_tensor("hb_scratch", [b_loc, HIDDEN], F)
            nc.sync.dma_start(hb_scratch[:, :], hb_nat[:].bitcast(F))
            hbT64 = const_pool.tile([64, 2 * HT, b_loc], R, tag="hbT64")
            with nc.allow_non_contiguous_dma("tiny preamble gather"):
                for b in range(b_loc):
                    nc.gpsimd.dma_start(
                        out=hbT64[:, :, b],
                        in_=hb_scratch[b, :]
                        .rearrange("(i p) -> p i", p=64)
                        .bitcast(R),
                    )

            # ---- per-chunk prepare: one chunk-granular load, one casting
            # DMA to fp8 scratch, 4 big DRAM-source XBAR transposes ----
            # enc rows indexed as s = c*S_CHUNK + t*128 + p
            enc_re = enc.rearrange(
                "b (c t p) e -> b c p t e", c=n_chunk, t=SUBS, p=128
            )
            def emit_prepare(b, c):
                nat = nat_pool.tile([128, SUBS, ENC], R, tag="nat", name="nat")
                nc.sync.dma_start(nat[:], enc_re[b, c, :, :, :])
                # fp32 -> fp8e4m3 on DVE; the pairing for DoubleRow is
                # (e, e+512), realized as separate e-tile planes of encT8.
                # (The XBAR DMA transpose and the gpsimd casting DMA both
                # corrupt data at full scale; PE fp8 transposes are safe and
                # ISA-legal, unlike uint16.)
                nat8 = nat8_pool.tile(
                    [128, SUBS, ENC], mybir.dt.float8e4, tag="nat8", name="nat8"
                )
                nc.vector.tensor_copy(nat8[:], nat[:].bitcast(F))
                return nat, nat8

            def emit_trans_unit(nat8, eT, t2, i):
                # One PE fp8 transpose group (1.0 c/r) + its DVE pair-copy.
                # HW writes fp8 transpose output at element step 2; e-tiles
                # t2 and t2+4 land in the byte-interleaved (e, e+512) pair
                # layout of eT via the strided copy.
                pt = tp_pool.tile(
                    [128, S_CHUNK, 2], mybir.dt.float8e4, tag="tp16", name="pt8"
                )
                if "transpose" not in ablate:
                    for t in range(SUBS):
                        nc.tensor.matmul(
                            pt[:, ts(t, 128), 0],
                            nat8[:, t, ts(t2 + 4 * i, 128)],
                            ident8[:],
                            is_transpose=True,
                            start=(t == 0),
                            stop=(t == SUBS - 1),
                        )
                nc.vector.tensor_copy(eT[:, :, i], pt[:, :, 0])

            def make_trans_units(nat8):
                encTs = [
                    encT_pool.tile(
                        [128, S_CHUNK, 2], mybir.dt.float8e4, tag="encT", name="eT"
                    )
                    for _ in range(ET2)
                ]
                units = [
                    (lambda nat8=nat8, eT=encTs[t2], t2=t2, i=i: emit_trans_unit(
                        nat8, eT, t2, i
                    ))
                    for t2 in range(ET2)
                    for i in range(2)
                ]
                return encTs, units

            # ---- deferred softmax/context closures ----
            def emit_exp_act(c, zparts, attn_ps, state):
                # ACT part of stage A: exp + softmax denominator
                if "ctx" in ablate or "attn" in ablate:
                    return
                exp_row = mid_pool.tile([2, S_CHUNK], R, tag="exp_row", name="exp_row")
                nc.scalar.activation(
                    exp_row[:],
                    attn_ps[:],
                    AF.Exp,
                    accum_out=zparts[0:2, c : c + 1],
                )
                if debug and state.get("dbg_b") == 0:
                    nc.sync.dma_start(dbg_exp[c, :, :], exp_row[:].bitcast(F))
                state["exp_row"] = exp_row

            def emit_exp_tp_single(state, t):
                # PE part of stage A: transpose one exp-row block to a
                # [s=128p] column (tp_pool has 1 buf; drains are spread)
                if "ctx" in ablate or "attn" in ablate:
                    state.setdefault("ecs", []).append(None)
                    return
                exp_row = state["exp_row"]
                pt = tp_pool.tile([128, 2], R, tag="tp16")
                nc.tensor.matmul(
                    pt[:],
                    exp_row[0:2, ts(t, 128)],
                    ident[0:2, 0:2],
                    is_transpose=True,
                    start=True,
                    stop=True,
                )
                ec = small_pool.tile([128, 2], R, tag="ec")
                nc.vector.tensor_copy(ec[:], pt[:])
                state.setdefault("ecs", []).append(ec)

            def emit_ctx_mms(c, ctx_ps, ecs, nat):
                # stage B: context accumulation against the natural enc tiles
                if "ctx" in ablate or "attn" in ablate:
                    return
                for t in range(SUBS):
                    for n in range(2):
                        nc.tensor.matmul(
                            ctx_ps[:, ts(n, 512)],
                            ecs[t][:, 0:1],
                            nat[:, t, ts(n, 512)],
                            start=(c == 0 and t == 0),
                            stop=(c == n_chunk - 1 and t == SUBS - 1),
                        )

            def emit_finalize(b, ctx_ps, zparts):
                if "ctx" in ablate or "attn" in ablate:
                    return
                zsum = small_pool.tile([1, 1], F, tag="zsum")
                nc.vector.tensor_reduce(
                    zsum[:],
                    zparts[0:1, :],
                    mybir.AxisListType.X,
                    mybir.AluOpType.add,
                )
                rz = small_pool.tile([1, 1], F, tag="rz")
                nc.vector.reciprocal(rz[:], zsum[:])
                ctx_sb = mid_pool.tile([1, ENC], F, tag="ctx_sb")
                nc.vector.tensor_scalar_mul(ctx_sb[:], ctx_ps[:], rz[:])
                nc.sync.dma_start(out[b : b + 1, :], ctx_sb[:])

            # ---- main loop ----
            def emit_main():
                pending_act = []  # exp ACT (no PE cost) — drain at chunk start
                pending_tp = []   # exp PE transposes + DVE copies
                pending_b = []    # ctx matmuls / finalize
                flat = [(b, c) for b in range(b_loc) for c in range(n_chunk)]
                look = T["lookahead"]
                preps = [emit_prepare(*flat[k]) for k in range(look)]
                # transposes for chunk 0 emitted inline (once per iteration)
                encT_fifo = []
                first_encTs, first_units = make_trans_units(preps[0][1])
                for fn in first_units:
                    fn()
                encT_fifo.append(first_encTs)
                pending_trans = []
                for b in range(b_loc):
                    ctx_ps = ctx_pool.tile([1, ENC], F, tag="ctx", name="ctx_ps")
                    zparts = small_pool.tile(
                        [2, n_chunk], F, tag="zparts", name="zparts"
                    )
                    for c in range(n_chunk):
                        nat, nat8 = preps.pop(0)
                        # exp ACT of previous chunk: ACT is idle, emit first
                        for fn in pending_act:
                            fn()
                        del pending_act[:]
                        # any transpose units for THIS chunk not yet emitted
                        for fn in pending_trans:
                            fn()
                        del pending_trans[:]
                        encTs = encT_fifo.pop(0)
                        # prepare a later chunk (load/cast/transposes run on
                        # SP/Pool/DMA queues while PE crunches this chunk)
                        k = b * n_chunk + c + look
                        if k < len(flat):
                            preps.append(emit_prepare(*flat[k]))
                        # queue next chunk's transpose units, spread over this
                        # chunk's h-loop (PE slack while ACT catches up)
                        if b * n_chunk + c + 1 < len(flat):
                            nxt_encTs, nxt_units = make_trans_units(preps[0][1])
                            encT_fifo.append(nxt_encTs)
                            pending_trans.extend(nxt_units)

                        attn_ps = attn_pool.tile(
                            [2, S_CHUNK], F, tag="attn", name="attn_ps"
                        )
                        lag = T["attn_lag"]
                        ets = []
                        for i in range(2 * HT):  # 16 h64-tiles
                            pm = main_pool.tile(
                                [64, S_CHUNK], F, tag="main", name="pm"
                            )
                            if "main" not in ablate:
                                for n in range(2):
                                    for t2 in range(ET2):
                                        rhs = encTs[t2][:].rearrange(
                                            "p s two -> p two s"
                                        )[:, :, ts(n, 256)]
                                        nc.tensor.matmul(
                                            pm[:, ts(n, 256)],
                                            w_e8[:, t2, :, ts(i, 64)],
                                            rhs,
                                            start=(t2 == 0),
                                            stop=(t2 == ET2 - 1),
                                            perf_mode=DR,
                                        )
                            et = energyT_pool.tile(
                                [64, S_CHUNK], R, tag="energyT", name="et"
                            )
                            if "tanh" not in ablate and "main" not in ablate:
                                nc.scalar.activation(
                                    et[:], pm[:], AF.Tanh, bias=hbT64[:, i, b : b + 1]
                                )
                            ets.append(et)
                            if "attn" not in ablate and i >= lag:
                                nc.tensor.matmul(
                                    attn_ps[:],
                                    vT64[:, i - lag, :],
                                    ets[i - lag][:],
                                    start=(i - lag == 0),
                                    stop=False,
                                )
                            if debug and b == 0 and c == 0:
                                nc.sync.dma_start(
                                    dbg_et[i, :, :], et[:].bitcast(F)
                                )
                            if (
                                i >= T["drain_exp_tp"]
                                and i < T["drain_exp_tp"] + SUBS
                                and pending_tp
                            ):
                                pending_tp.pop(0)()
                            if i >= 8 and pending_trans:
                                pending_trans.pop(0)()
                            if i == T["drain_ctx"]:
                                for fn in pending_b:
                                    fn()
                                del pending_b[:]
                        if "attn" not in ablate:
                            for i in range(2 * HT - lag, 2 * HT):
                                nc.tensor.matmul(
                                    attn_ps[:],
                                    vT64[:, i, :],
                                    ets[i][:],
                                    start=False,
                                    stop=(i == 2 * HT - 1),
                                )
                        if debug and b == 0:
                            at_sb = mid_pool.tile(
                                [2, S_CHUNK], F, tag="at_sb", name="at_sb"
                            )
                            nc.vector.tensor_copy(at_sb[:], attn_ps[:])
                            nc.sync.dma_start(dbg_attn[c, :, :], at_sb[:])
                        if debug and b == 0 and c == 0:
                            for t2 in range(ET2):
                                nc.sync.dma_start(
                                    dbg_encT_late[t2, :, :], encTs[t2][:]
                                )
                        state = {"dbg_b": b}
                        pending_act.append(
                            lambda c=c, zparts=zparts, attn_ps=attn_ps, state=state: (
                                emit_exp_act(c, zparts, attn_ps, state)
                            )
                        )
                        for t in range(SUBS):
                            pending_tp.append(
                                lambda state=state, t=t: emit_exp_tp_single(state, t)
                            )
                        pending_b.append(
                            lambda c=c, ctx_ps=ctx_ps, nat=nat, state=state: (
                                emit_ctx_mms(c, ctx_ps, state["ecs"], nat)
                            )
                        )
                    pending_b.append(
                        lambda b=b, ctx_ps=ctx_ps, zparts=zparts: emit_finalize(
                            b, ctx_ps, zparts
                        )
                    )
                for fn in pending_act:
                    fn()
                del pending_act[:]
                for fn in pending_tp:
                    fn()
                del pending_tp[:]
                for fn in pending_b:
                    fn()
                del pending_b[:]

            if repeat > 1:
                with tc.For_i(0, repeat, 1):
                    emit_main()
            else:
                emit_main()

    nc.compile()
    return nc


def kernel_run(hidden, encoder_outputs, attn_w, attn_b, v_w, **spmd_kwargs):
    """Shards over batch across 8 cores, runs the Bass kernel SPMD, gathers
    per-core outputs. Returns (full_output, BassKernelResults)."""
    global _CACHED_NC
    from concourse.bass_utils import run_bass_kernel_spmd

    if _CACHED_NC is None:
        _CACHED_NC = build_bass()
    nc = _CACHED_NC

    hidden = np.asarray(hidden, dtype=np.float32).reshape(BATCH, HIDDEN)
    enc = np.ascontiguousarray(np.asarray(encoder_outputs, dtype=np.float32))
    attn_w = np.ascontiguousarray(np.asarray(attn_w, dtype=np.float32))
    attn_b = np.ascontiguousarray(np.asarray(attn_b, dtype=np.float32))
    v_w = np.ascontiguousarray(np.asarray(v_w, dtype=np.float32))

    in_maps = []
    for c in range(NCORES):
        lo, hi = c * B_LOC, (c + 1) * B_LOC
        in_maps.append(
            {
                "hidden": np.ascontiguousarray(hidden[lo:hi]),
                "enc": np.ascontiguousarray(enc[lo:hi]),
                "attn_w": attn_w,
                "attn_b": attn_b,
                "v_w": v_w,
            }
        )

    res = run_bass_kernel_spmd(
        nc, in_maps, core_ids=list(range(NCORES)), **spmd_kwargs
    )
    outs = [r["out"] for r in res.results]
    full = np.concatenate(outs, axis=0).reshape(BATCH, 1, ENC)
    return full, res


def kernel(hidden, encoder_outputs, attn_w, attn_b, v_w):
    """Full-input entry point: takes the full (unsharded) inputs, returns the
    full [32, 1, 1024] output."""
    full, _ = kernel_run(hidden, encoder_outputs, attn_w, attn_b, v_w)
    return full



# revision 4
# speedup vs baseline: 1.1260x; 1.1260x over previous
"""Trainium2 Bass kernel for nn_Attention (Bahdanau-style attention decode step).

Reference computation (per batch b):
    h_proj  = hidden[b] @ W_h                      # [1, H]
    e_proj  = enc[b] @ W_e                         # [S, H]
    energy  = tanh(e_proj + h_proj + bias)         # [S, H]
    attn    = energy @ v                           # [S]
    w       = softmax(attn)                        # [S]
    context = w @ enc[b]                           # [E]

Sharding: data-parallel over batch on 8 cores (4 batches/core), no collectives.

Per-core kernel strategy (full-width DoubleRow, scaled fp8, fp16 pair
transposes, fp8 attn, interleaved ctx):
  - enc tiles DMA'd in natural [s=128p, e] layout; kept fp32 for the context
    matmul's accuracy. fp32->fp8e4m3 cast is written PAIR-PACKED (uint16
    cell = (e, e+512) DoubleRow pair), split across GPSIMD and DVE (one
    byte-lane each) so neither engine bottlenecks.
  - PE fp16 pair-cell transposes: is_transpose is a bit-exact pass-through
    (HW-verified incl NaN/denormal patterns), so transposing the fp16 VIEW
    of the packed fp8 pairs moves both lanes at once — half the PE transpose
    rows of the fp8 path, and the PSUM->SBUF evacuation becomes a contiguous
    uint16 copy eligible for the DVE 2x mode.
  - W_e is stored fp8 scaled by x32: raw W_e ~ U(+-0.022) lands 71% in the
    fp8e4m3 subnormal range; scaling recovers the full 3-bit mantissa
    (rel_l2 1.55e-2 -> 1.10e-2). The 1/32 compensation is free via the tanh
    ACT scale operand.
  - Main matmul: stationary w_e8[:, t2, :, 128h-block] is [128, 2, 128]
    (FULL 128-wide output — the original used 64 and idled half the array),
    moving is the pair view of encT [128, 2, 512], out is a full [128, 512]
    PSUM bank. 4 accumulation MMs per h128-block, 32 per chunk. HW-measured
    213.9 ns/MM — LDWEIGHTS-bound (256-col DR stationary load at 1.2 GHz).
  - tanh on ACT (scale=1/32, per-partition bias hbT128[:, i, b] folding
    h_proj+bias) writes energy DIRECTLY as fp8 into byte-interleaved pair
    tiles (lane i%2 of pair i//2).
  - attention on PE in fp8 DoubleRow: stationary v8p[128, 2, 2] holds
    (v[2j*128+p], v[(2j+1)*128+p]) pairs scaled x256 (subnormal avoidance),
    moving is the pair view of the energy tile; 4 MMs of N=512 per chunk.
    The 1/256 compensation is free via the exp ACT scale operand.
  - exp on ACT (scale=1/256) with accum_out accumulating the softmax
    denominator.
  - exp rows transposed back to [s=128p] columns on PE (tiny); context
    rank-1 updates against the natural-layout fp32 enc tiles are
    INTERLEAVED one-two per h-tile so their 213ns moving streams hide the
    next main MM's DR stationary load.
  - Pipeline: chunk c+1's load/cast/transpose chain is emitted during chunk
    c's compute; the exp/softmax/context chain for chunk c is drained inside
    chunk c+1's h-loop so PE never head-blocks.
"""

import numpy as np

HIDDEN = 1024
ENC = 1024
BATCH = 32
SEQ = 2048
NCORES = 8
B_LOC = BATCH // NCORES  # 4

S_CHUNK = 512
N_CHUNK = SEQ // S_CHUNK  # 4
SUBS = S_CHUNK // 128  # 4 s-subtiles per chunk
ET = ENC // 128  # 8 e-tiles
ET2 = ENC // 256  # 4 e-pair-tiles (DoubleRow)
HT = HIDDEN // 128  # 8 h-tiles
HT2 = HT // 2  # 4 h-pair-tiles (DoubleRow attn)

W_SCALE = 32.0  # W_e fp8 pre-scale (subnormal avoidance)
V_SCALE = 256.0  # v fp8 pre-scale

_CACHED_NC = None


def build_bass(b_loc=B_LOC, seq=SEQ, repeat=1, ablate=(), tune=None, debug=False):
    ablate = set(ablate)
    T = {
        "drain_exp_tp": 2,  # h128-tile index: first exp-transpose drain (4 singles)
        "ctx_start": 2,     # h128-tile index: first interleaved ctx MM
        "ctx_per_i": 2,     # ctx MMs drained per h128-tile
        "drain_ctx": 7,     # h128-tile index: earliest finalize drain
        "attn_lag": 2,      # h128-tiles between a pair's 2nd tanh and its attn MM
        "lookahead": 2,     # chunks prepared ahead of compute
        "trans_start": 4,   # h128-tile index to start draining next transposes
        "trans_per_i": 1,   # transpose units drained per h128-tile
        "conv": "split",    # fp32->fp8 cast: "pool" | "dve" | "split" (one lane each)
        "const_stationary": 0,  # DEBUG: reuse one stationary for all main MMs
        "nat_bufs": 4,
        "encT_bufs": 8,
        "energyT_bufs": 6,
        "main_bufs": 2,
        "tp_bufs": 2,
        "attn_bufs": 2,
    }
    T.update(tune or {})
    import concourse.mybir as mybir
    import concourse.tile as tile
    from concourse import bacc
    from concourse.bass import ts
    from concourse.masks import make_identity

    n_chunk = seq // S_CHUNK

    nc = bacc.Bacc()
    R = mybir.dt.float32r
    F = mybir.dt.float32
    F8 = mybir.dt.float8e4
    F16 = mybir.dt.float16
    U16 = mybir.dt.uint16
    AF = mybir.ActivationFunctionType
    DR = mybir.MatmulPerfMode.DoubleRow

    hidden = nc.dram_tensor("hidden", [b_loc, HIDDEN], R, kind="ExternalInput")
    enc = nc.dram_tensor("enc", [b_loc, seq, ENC], R, kind="ExternalInput")
    attn_w = nc.dram_tensor("attn_w", [HIDDEN + ENC, HIDDEN], R, kind="ExternalInput")
    attn_b = nc.dram_tensor("attn_b", [HIDDEN], F, kind="ExternalInput")
    v_w = nc.dram_tensor("v_w", [HIDDEN], R, kind="ExternalInput")
    out = nc.dram_tensor("out", [b_loc, ENC], F, kind="ExternalOutput")
    if debug:
        n_chunk_dbg = seq // S_CHUNK
        dbg_attn = nc.dram_tensor(
            "dbg_attn", [n_chunk_dbg, 2, S_CHUNK], F, kind="ExternalOutput"
        )
        dbg_exp = nc.dram_tensor(
            "dbg_exp", [n_chunk_dbg, 2, S_CHUNK], F, kind="ExternalOutput"
        )

    with tile.TileContext(nc) as tc:
        with (
            tc.tile_pool(name="weights", bufs=1) as w_pool,
            tc.tile_pool(name="consts", bufs=1) as const_pool,
            tc.tile_pool(name="nat", bufs=T["nat_bufs"]) as nat_pool,
            tc.tile_pool(name="nat8", bufs=3) as nat8_pool,
            tc.tile_pool(name="encT", bufs=T["encT_bufs"]) as encT_pool,
            tc.tile_pool(name="energyT", bufs=T["energyT_bufs"]) as energyT_pool,
            tc.tile_pool(name="small", bufs=8) as small_pool,
            tc.tile_pool(name="mid", bufs=3) as mid_pool,
            tc.tile_pool(name="ps_tp", bufs=T["tp_bufs"], space="PSUM") as tp_pool,
            tc.tile_pool(name="ps_main", bufs=T["main_bufs"], space="PSUM") as main_pool,
            tc.tile_pool(name="ps_attn", bufs=T["attn_bufs"], space="PSUM") as attn_pool,
            tc.tile_pool(name="ps_ctx", bufs=1, space="PSUM") as ctx_pool,
        ):
            # ---- constants / weights ----
            ident_f = const_pool.tile([128, 128], F, tag="ident_f")
            make_identity(nc, ident_f[:])
            ident = const_pool.tile([128, 128], R)
            nc.vector.tensor_copy(ident[:], ident_f[:])
            ident16 = const_pool.tile([128, 128], F16, tag="ident16")
            nc.vector.tensor_copy(ident16[:], ident_f[:])


            w_h = w_pool.tile([128, ET, HIDDEN], R, tag="w_h")
            nc.sync.dma_start(
                w_h[:], attn_w[0:HIDDEN, :].rearrange("(j p) h -> p j h", p=128)
            )

            # W_e in fp8 with (e, e+512) pairing for DoubleRow:
            #   w_e8[p, t2, i, h] = W_e[i*512 + t2*128 + p, h]
            w_e8 = w_pool.tile([128, ET2, 2, HIDDEN], F8, tag="w_e8")
            we_re = attn_w[HIDDEN : HIDDEN + ENC, :].rearrange(
                "(two t p) h -> p t two h", two=2, t=ET2, p=128
            )
            for t2 in range(ET2):
                wstage = w_pool.tile([128, 2, HIDDEN], R, tag="wstage", name="wstage")
                nc.sync.dma_start(wstage[:], we_re[:, t2, :, :])
                nc.vector.tensor_scalar_mul(
                    w_e8[:, t2, :, :], wstage[:].bitcast(F), W_SCALE
                )

            # ones row for rank-1 partition-broadcast matmuls
            ones_f = const_pool.tile([1, 128], F, tag="ones_f")
            nc.vector.memset(ones_f[:], 1.0)
            ones_r = const_pool.tile([1, 128], R, tag="ones_r")
            nc.vector.tensor_copy(ones_r[:], ones_f[:])

            # v as fp8 DoubleRow pairs, scaled x256 (subnormal avoidance):
            #   v8p[p, j, plane, col] = fp8(256 * v[(2j+plane)*128 + p]),
            # duplicated into cols {0,1} so the attention row lands on 2 PSUM
            # partitions. The plane stride is padded to 16B (ISA step%16==0).
            vstage = const_pool.tile([128, HT], R, tag="vstage")
            nc.gpsimd.dma_start(
                out=vstage[:], in_=v_w[:].rearrange("(i p) -> p i", p=128)
            )
            vsc = const_pool.tile([128, HT], F, tag="vsc")
            nc.vector.tensor_scalar_mul(vsc[:], vstage[:].bitcast(F), V_SCALE)
            v8p = const_pool.tile([128, HT2, 2, 16], F8, tag="v8p")
            for j in range(HT2):
                for plane in range(2):
                    for col in range(2):
                        nc.vector.tensor_copy(
                            v8p[:, j, plane, col : col + 1],
                            vsc[:, 2 * j + plane : 2 * j + plane + 1],
                        )

            # ---- preamble: hb[b, h] = hidden[b] @ W_h + attn_b, laid out as
            # hbT128 [h=128p, i, b] per-partition bias columns ----
            # NOTE: PE fp32r transposes ahead of DoubleRow matmuls corrupt the
            # DR moving fetch on HW (first 4 of every 16 s-columns), so all
            # preamble transposes are done as (slow, but loop-external)
            # non-contiguous DMA gathers instead.
            hiddenT = const_pool.tile([128, ET, b_loc], R, tag="hiddenT")
            with nc.allow_non_contiguous_dma("tiny preamble gather"):
                for b in range(b_loc):
                    nc.gpsimd.dma_start(
                        out=hiddenT[:, :, b],
                        in_=hidden[b, :].rearrange("(j p) -> p j", p=128),
                    )

            attnb_sb = const_pool.tile([b_loc, HIDDEN], F, tag="attnb")
            nc.gpsimd.dma_start(
                out=attnb_sb[:], in_=attn_b[:].partition_broadcast(b_loc)
            )
            hb_ps = ctx_pool.tile([b_loc, HIDDEN], F, tag="ctx")
            for n in range(2):
                for j in range(ET):
                    nc.tensor.matmul(
                        hb_ps[:, ts(n, 512)],
                        hiddenT[:, j, :],
                        w_h[:, j, ts(n, 512)],
                        start=(j == 0),
                        stop=(j == ET - 1),
                    )
            hb_nat = const_pool.tile([b_loc, HIDDEN], R, tag="hb_nat")
            nc.vector.tensor_add(hb_nat[:], hb_ps[:], attnb_sb[:])

            # per-h128-tile bias columns (DR outputs now span all 128
            # partitions); via DRAM roundtrip + non-contiguous gather, not PE
            # transposes (see note above)
            hb_scratch = nc.dram_tensor("hb_scratch", [b_loc, HIDDEN], F)
            nc.sync.dma_start(hb_scratch[:, :], hb_nat[:].bitcast(F))
            hbT128 = const_pool.tile([128, HT, b_loc], R, tag="hbT128")
            with nc.allow_non_contiguous_dma("tiny preamble gather"):
                for b in range(b_loc):
                    nc.gpsimd.dma_start(
                        out=hbT128[:, :, b],
                        in_=hb_scratch[b, :]
                        .rearrange("(i p) -> p i", p=128)
                        .bitcast(R),
                    )

            # ---- per-chunk prepare: one chunk-granular load + DVE cast ----
            # enc rows indexed as s = c*S_CHUNK + t*128 + p
            enc_re = enc.rearrange(
                "b (c t p) e -> b c p t e", c=n_chunk, t=SUBS, p=128
            )
            def emit_prepare(b, c):
                nat = nat_pool.tile([128, SUBS, ENC], R, tag="nat", name="nat")
                nc.sync.dma_start(nat[:], enc_re[b, c, :, :, :])
                # fp32 -> fp8e4m3, written PAIR-PACKED: nat8p[s, t, q, lane]
                # = fp8(enc[s, t-block, lane*512 + q]), so each uint16 cell
                # holds one (e, e+512) DoubleRow contraction pair. Two
                # strided casts (GPSIMD by default; DVE is busier).
                nat8p = nat8_pool.tile(
                    [128, SUBS, ENC // 2, 2],
                    mybir.dt.float8e4,
                    tag="nat8",
                    name="nat8p",
                )
                if T["conv"] == "split":
                    engs = [nc.gpsimd, nc.vector]
                else:
                    eng = nc.gpsimd if T["conv"] == "pool" else nc.vector
                    engs = [eng, eng]
                for lane in range(2):
                    engs[lane].tensor_copy(
                        nat8p[:, :, :, lane],
                        nat[:, :, ts(lane, ENC // 2)].bitcast(F),
                    )
                return nat, nat8p

            def emit_trans_unit(nat8p, eT, t2):
                # One PE fp16 pair-cell transpose group: each fp16 "value" is
                # a (e, e+512) byte-interleaved fp8 pair; is_transpose is a
                # bit-exact pass-through (HW-verified incl NaN/denormal
                # patterns), so transposing the fp16 VIEW moves both fp8
                # lanes at once — half the PE rows and a contiguous 2-byte
                # PSUM->SBUF copy (DVE 2x mode) instead of strided fp8.
                pt = tp_pool.tile([128, S_CHUNK], F16, tag="tp16", name="pt16")
                if "transpose" not in ablate:
                    for t in range(SUBS):
                        nc.tensor.matmul(
                            pt[:, ts(t, 128)],
                            nat8p[:, t, ts(t2, 128), :].bitcast(F16),
                            ident16[:],
                            is_transpose=True,
                            start=(t == 0),
                            stop=(t == SUBS - 1),
                        )
                nc.vector.tensor_copy(
                    eT[:].bitcast(U16), pt[:].bitcast(U16)
                )

            def make_trans_units(nat8p):
                encTs = [
                    encT_pool.tile(
                        [128, S_CHUNK, 2], mybir.dt.float8e4, tag="encT", name="eT"
                    )
                    for _ in range(ET2)
                ]
                units = [
                    (lambda nat8p=nat8p, eT=encTs[t2], t2=t2: emit_trans_unit(
                        nat8p, eT, t2
                    ))
                    for t2 in range(ET2)
                ]
                return encTs, units

            # ---- deferred softmax/context closures ----
            def emit_exp_act(c, zparts, attn_ps, state):
                # ACT part of stage A: exp + softmax denominator
                if "ctx" in ablate or "attn" in ablate:
                    return
                exp_row = mid_pool.tile([2, S_CHUNK], R, tag="exp_row", name="exp_row")
                # attn_ps holds 256*attention (v scaled x256); undo via scale
                nc.scalar.activation(
                    exp_row[:],
                    attn_ps[:],
                    AF.Exp,
                    scale=1.0 / V_SCALE,
                    accum_out=zparts[0:2, c : c + 1],
                )
                if debug and state.get("dbg_b") == 0:
                    nc.sync.dma_start(dbg_exp[c, :, :], exp_row[:].bitcast(F))
                state["exp_row"] = exp_row

            def emit_exp_tp_single(state, t):
                # PE part of stage A: transpose one exp-row block to a
                # [s=128p] column (tp_pool has 1 buf; drains are spread)
                if "ctx" in ablate or "attn" in ablate:
                    state.setdefault("ecs", []).append(None)
                    return
                exp_row = state["exp_row"]
                pt = tp_pool.tile([128, 2], R, tag="tp16")
                nc.tensor.matmul(
                    pt[:],
                    exp_row[0:2, ts(t, 128)],
                    ident[0:2, 0:2],
                    is_transpose=True,
                    start=True,
                    stop=True,
                )
                ec = small_pool.tile([128, 2], R, tag="ec")
                nc.vector.tensor_copy(ec[:], pt[:])
                state.setdefault("ecs", []).append(ec)

            def emit_ctx_single(c, ctx_ps, state, nat, t, n):
                # stage B: one rank-1 context update. Interleaved between
                # main-MM groups so its 213ns moving stream hides the next
                # DR stationary load (the main matmul is LDWEIGHTS-bound:
                # 256-col DR load = 213ns vs 107ns stream). Col-tiling via
                # tile_position is rejected by this walrus for col>0.
                if "ctx" in ablate or "attn" in ablate:
                    return
                nc.tensor.matmul(
                    ctx_ps[:, ts(n, 512)],
                    state["ecs"][t][:, 0:1],
                    nat[:, t, ts(n, 512)],
                    start=(c == 0 and t == 0),
                    stop=(c == n_chunk - 1 and t == SUBS - 1),
                )

            def emit_finalize(b, ctx_ps, zparts):
                if "ctx" in ablate or "attn" in ablate:
                    return
                zsum = small_pool.tile([1, 1], F, tag="zsum")
                nc.vector.tensor_reduce(
                    zsum[:],
                    zparts[0:1, :],
                    mybir.AxisListType.X,
                    mybir.AluOpType.add,
                )
                rz = small_pool.tile([1, 1], F, tag="rz")
                nc.vector.reciprocal(rz[:], zsum[:])
                ctx_sb = mid_pool.tile([1, ENC], F, tag="ctx_sb")
                nc.vector.tensor_scalar_mul(ctx_sb[:], ctx_ps[:], rz[:])
                nc.sync.dma_start(out[b : b + 1, :], ctx_sb[:])

            # ---- main loop ----
            def emit_main():
                pending_act = []  # exp ACT (no PE cost) — drain at chunk start
                pending_tp = []   # exp PE transposes + DVE copies
                pending_ctx = []  # single ctx MMs, interleaved into the h-loop
                pending_b = []    # finalize
                flat = [(b, c) for b in range(b_loc) for c in range(n_chunk)]
                look = T["lookahead"]
                preps = [emit_prepare(*flat[k]) for k in range(look)]
                # transposes for chunk 0 emitted inline (once per iteration)
                encT_fifo = []
                first_encTs, first_units = make_trans_units(preps[0][1])
                for fn in first_units:
                    fn()
                encT_fifo.append(first_encTs)
                pending_trans = []
                for b in range(b_loc):
                    ctx_ps = ctx_pool.tile([1, ENC], F, tag="ctx", name="ctx_ps")
                    zparts = small_pool.tile(
                        [2, n_chunk], F, tag="zparts", name="zparts"
                    )
                    for c in range(n_chunk):
                        nat, nat8 = preps.pop(0)
                        # exp ACT of previous chunk: ACT is idle, emit first
                        for fn in pending_act:
                            fn()
                        del pending_act[:]
                        # any transpose units for THIS chunk not yet emitted
                        for fn in pending_trans:
                            fn()
                        del pending_trans[:]
                        encTs = encT_fifo.pop(0)
                        # prepare a later chunk (load/cast run on
                        # SP/Pool/DMA queues while PE crunches this chunk)
                        k = b * n_chunk + c + look
                        if k < len(flat):
                            preps.append(emit_prepare(*flat[k]))
                        # queue next chunk's transpose units, spread over this
                        # chunk's h-loop (PE slack while ACT catches up)
                        if b * n_chunk + c + 1 < len(flat):
                            nxt_encTs, nxt_units = make_trans_units(preps[0][1])
                            encT_fifo.append(nxt_encTs)
                            pending_trans.extend(nxt_units)

                        attn_ps = attn_pool.tile(
                            [2, S_CHUNK], F, tag="attn", name="attn_ps"
                        )
                        lag = T["attn_lag"]
                        etps = []  # fp8 energy pair tiles, one per j = i//2

                        def emit_attn_pair(j, etps=None):
                            # fp8 DoubleRow: contraction over 256 h at once
                            rhs = etps[j][:].rearrange("p s two -> p two s")
                            nc.tensor.matmul(
                                attn_ps[:],
                                v8p[:, j, :, 0:2],
                                rhs,
                                start=(j == 0),
                                stop=(j == HT2 - 1),
                                perf_mode=DR,
                            )

                        next_attn = [0]
                        for i in range(HT):  # 8 h128-tiles
                            pm = main_pool.tile(
                                [128, S_CHUNK], F, tag="main", name="pm"
                            )
                            if "main" not in ablate:
                                for t2 in range(ET2):
                                    rhs = encTs[t2][:].rearrange(
                                        "p s two -> p two s"
                                    )
                                    if T["const_stationary"]:
                                        lhsT = w_e8[:, 0, :, ts(0, 128)]
                                    else:
                                        lhsT = w_e8[:, t2, :, ts(i, 128)]
                                    nc.tensor.matmul(
                                        pm[:],
                                        lhsT,
                                        rhs,
                                        start=(t2 == 0),
                                        stop=(t2 == ET2 - 1),
                                        perf_mode=DR,
                                    )
                            if i % 2 == 0:
                                etps.append(
                                    energyT_pool.tile(
                                        [128, S_CHUNK, 2],
                                        F8,
                                        tag="energyT",
                                        name="etp",
                                    )
                                )
                            if "tanh" not in ablate and "main" not in ablate:
                                # energy straight to fp8, byte-interleaved
                                # into lane i%2 of pair i//2; x32 W_e scale
                                # undone by the free ACT scale operand
                                nc.scalar.activation(
                                    etps[i // 2][:, :, i % 2],
                                    pm[:],
                                    AF.Tanh,
                                    bias=hbT128[:, i, b : b + 1],
                                    scale=1.0 / W_SCALE,
                                )
                            if "attn" not in ablate:
                                while (
                                    next_attn[0] < HT2
                                    and 2 * next_attn[0] + 1 + lag <= i
                                ):
                                    emit_attn_pair(next_attn[0], etps)
                                    next_attn[0] += 1
                            if (
                                i >= T["drain_exp_tp"]
                                and i < T["drain_exp_tp"] + 2
                                and pending_tp
                            ):
                                pending_tp.pop(0)()
                                if pending_tp:
                                    pending_tp.pop(0)()
                            if i >= T["ctx_start"]:
                                for _ in range(T["ctx_per_i"]):
                                    if pending_ctx:
                                        pending_ctx.pop(0)()
                            if i >= T["trans_start"]:
                                for _ in range(T["trans_per_i"]):
                                    if pending_trans:
                                        pending_trans.pop(0)()
                            if i >= T["drain_ctx"] and not pending_ctx:
                                for fn in pending_b:
                                    fn()
                                del pending_b[:]
                        if "attn" not in ablate:
                            while next_attn[0] < HT2:
                                emit_attn_pair(next_attn[0], etps)
                                next_attn[0] += 1
                        if debug and b == 0:
                            at_sb = mid_pool.tile(
                                [2, S_CHUNK], F, tag="at_sb", name="at_sb"
                            )
                            nc.vector.tensor_copy(at_sb[:], attn_ps[:])
                            nc.sync.dma_start(dbg_attn[c, :, :], at_sb[:])
                        state = {"dbg_b": b}
                        pending_act.append(
                            lambda c=c, zparts=zparts, attn_ps=attn_ps, state=state: (
                                emit_exp_act(c, zparts, attn_ps, state)
                            )
                        )
                        for t in range(SUBS):
                            pending_tp.append(
                                lambda state=state, t=t: emit_exp_tp_single(state, t)
                            )
                        for t in range(SUBS):
                            for n in range(2):
                                pending_ctx.append(
                                    lambda c=c, ctx_ps=ctx_ps, nat=nat,
                                    state=state, t=t, n=n: emit_ctx_single(
                                        c, ctx_ps, state, nat, t, n
                                    )
                                )
                    pending_b.append(
                        lambda b=b, ctx_ps=ctx_ps, zparts=zparts: emit_finalize(
                            b, ctx_ps, zparts
                        )
                    )
                for fn in pending_act:
                    fn()
                del pending_act[:]
                for fn in pending_tp:
                    fn()
                del pending_tp[:]
                for fn in pending_ctx:
                    fn()
                del pending_ctx[:]
                for fn in pending_b:
                    fn()
                del pending_b[:]

            if repeat > 1:
                with tc.For_i(0, repeat, 1):
                    emit_main()
            else:
                emit_main()

    nc.compile()
    return nc


def kernel_run(hidden, encoder_outputs, attn_w, attn_b, v_w, **spmd_kwargs):
    """Shards over batch across 8 cores, runs the Bass kernel SPMD, gathers
    per-core outputs. Returns (full_output, BassKernelResults)."""
    global _CACHED_NC
    from concourse.bass_utils import run_bass_kernel_spmd

    if _CACHED_NC is None:
        _CACHED_NC = build_bass()
    nc = _CACHED_NC

    hidden = np.asarray(hidden, dtype=np.float32).reshape(BATCH, HIDDEN)
    enc = np.ascontiguousarray(np.asarray(encoder_outputs, dtype=np.float32))
    attn_w = np.ascontiguousarray(np.asarray(attn_w, dtype=np.float32))
    attn_b = np.ascontiguousarray(np.asarray(attn_b, dtype=np.float32))
    v_w = np.ascontiguousarray(np.asarray(v_w, dtype=np.float32))

    in_maps = []
    for c in range(NCORES):
        lo, hi = c * B_LOC, (c + 1) * B_LOC
        in_maps.append(
            {
                "hidden": np.ascontiguousarray(hidden[lo:hi]),
                "enc": np.ascontiguousarray(enc[lo:hi]),
                "attn_w": attn_w,
                "attn_b": attn_b,
                "v_w": v_w,
            }
        )

    res = run_bass_kernel_spmd(
        nc, in_maps, core_ids=list(range(NCORES)), **spmd_kwargs
    )
    outs = [r["out"] for r in res.results]
    full = np.concatenate(outs, axis=0).reshape(BATCH, 1, ENC)
    return full, res


def kernel(hidden, encoder_outputs, attn_w, attn_b, v_w):
    """Full-input entry point: takes the full (unsharded) inputs, returns the
    full [32, 1, 1024] output."""
    full, _ = kernel_run(hidden, encoder_outputs, attn_w, attn_b, v_w)
    return full
